# revision 1
# baseline (speedup 1.0000x reference)
"""MFDWC feature extractor as a Bass/Tile kernel for TRN2 (8 NeuronCores).

Pipeline (per batch row): pre-emphasis -> framing (999 frames x 882 samples,
hop 441) -> Hamming window -> rFFT(2048) power spectrum -> mel (60) -> log ->
Haar DWT -> delta -> mean/std over time -> 180 features.

Device mapping:
  - Data parallel: 16 batch rows -> 2 rows per core on 8 cores.
  - rFFT is computed as two DFT matmuls (cos / sin matrices, window folded in)
    in fp16 at full PE rate, fp32 PSUM accumulation.
  - The waveform is reshaped on-chip to put the sample-within-frame axis on
    SBUF partitions (PE transposes of 441-wide chunks); frames then appear as
    overlapping column views of a single (441, 1000) buffer.
  - Bins packing: cos matmul covers bins 0..1023; the sin matrix's bin-0
    column (which would be all zeros) instead carries the Nyquist cos column,
    and the two mel matrices are adjusted to match, so all 1025 power bins are
    covered by 2x1024 columns with no waste.
  - power -> mel is a second (tiny) matmul pair; log/Haar/delta/stats run on
    ACT/DVE engines.
"""

import math
from contextlib import ExitStack

import numpy as np

import concourse.bass as bass
import concourse.bacc as bacc
import concourse.mybir as mybir
import concourse.tile as tile
from concourse.bass_utils import run_bass_kernel_spmd

F32 = mybir.dt.float32
F16 = mybir.dt.float16
AF = mybir.ActivationFunctionType

B = 16               # batch
L = 441000           # samples per row
W = 441              # hop; also chunk width
NK = 1000            # number of 441-sample chunks per row (441*1000 = L)
FRAME = 882          # frame length
T = 999              # frames per row
NB = 1024            # matmul bins (bins 0..1023; Nyquist packed into sin col 0)
NMEL = 60
ROWS = 2             # batch rows per core
EPS = 1e-10
SQRT2 = math.sqrt(2.0)

# contraction chunks over the 882 frame samples: (r0, size, a) where the
# frame-sample index j = 441*a + r0 + i
KCH = [(0, 128, 0), (128, 128, 0), (256, 128, 0), (384, 57, 0),
       (0, 128, 1), (128, 128, 1), (256, 128, 1), (384, 57, 1)]
# chunks over the NK=1000 waveform rows
ECH = [(k * 128, min(128, NK - k * 128)) for k in range(8)]
# transpose row-blocks over the 441 samples per chunk
RBL = [(0, 128), (128, 128), (256, 128), (384, 57)]
# frame chunks (PSUM free-dim <= 512 fp32)
FCH = [(0, 512), (512, 487)]


def _host_constants(mel_filters: np.ndarray):
    """DFT / mel matrices with window folded in (fp16)."""
    j = np.arange(FRAME, dtype=np.float64)
    b = np.arange(NB, dtype=np.float64)
    ham = np.hamming(FRAME).astype(np.float64)
    ang = 2.0 * np.pi * np.outer(j, b) / 2048.0
    cw = (ham[:, None] * np.cos(ang)).astype(np.float16)          # (882, 1024)
    sw = ham[:, None] * np.sin(ang)
    sw[:, 0] = ham * np.cos(np.pi * j)                            # Nyquist cos col
    sw = sw.astype(np.float16)                                    # (882, 1024)
    m = mel_filters.astype(np.float64)                            # (60, 1025)
    mat = m[:, 0:NB].T.astype(np.float16)                         # (1024, 60)
    mbt = np.concatenate([m[:, NB:NB + 1], m[:, 1:NB]], axis=1).T.astype(np.float16)
    idn = np.eye(128, dtype=np.float16)
    hsum = np.zeros((NMEL, 30), np.float16)
    hdif = np.zeros((NMEL, 30), np.float16)
    for i in range(30):
        hsum[2 * i, i] = 1.0
        hsum[2 * i + 1, i] = 1.0
        hdif[2 * i, i] = 1.0
        hdif[2 * i + 1, i] = -1.0
    return cw, sw, mat, mbt, idn, hsum, hdif


def _body(ctx: ExitStack, tc, xpad, cw_d, sw_d, mat_d, mbt_d, idn_d, hs_d, hd_d, out_d):
    nc = tc.nc

    const = ctx.enter_context(tc.tile_pool(name="const", bufs=1))
    e2p = ctx.enter_context(tc.tile_pool(name="e2", bufs=3))
    emphp = ctx.enter_context(tc.tile_pool(name="emph", bufs=3))
    etp = ctx.enter_context(tc.tile_pool(name="et", bufs=1))
    ptrp = ctx.enter_context(tc.tile_pool(name="ptr", bufs=1, space="PSUM"))
    dftp = ctx.enter_context(tc.tile_pool(name="dft", bufs=2, space="PSUM"))
    melp = ctx.enter_context(tc.tile_pool(name="mel", bufs=1, space="PSUM"))
    haarp = ctx.enter_context(tc.tile_pool(name="haar", bufs=1, space="PSUM"))
    ppp = ctx.enter_context(tc.tile_pool(name="pp", bufs=2))
    lmp = ctx.enter_context(tc.tile_pool(name="lm", bufs=1))
    hop = ctx.enter_context(tc.tile_pool(name="ho", bufs=1))
    stp = ctx.enter_context(tc.tile_pool(name="st", bufs=2))

    # constants
    cw_t, sw_t = [], []
    for ki, (r0, sz, a) in enumerate(KCH):
        j0 = 441 * a + r0
        t = const.tile([128, NB], F16, tag=f"cw{ki}", name=f"cw{ki}")
        nc.sync.dma_start(t[0:sz, :], cw_d[j0:j0 + sz, :])
        cw_t.append(t)
        t = const.tile([128, NB], F16, tag=f"sw{ki}", name=f"sw{ki}")
        nc.sync.dma_start(t[0:sz, :], sw_d[j0:j0 + sz, :])
        sw_t.append(t)
    mat_t, mbt_t = [], []
    for c in range(8):
        t = const.tile([128, NMEL], F16, tag=f"ma{c}", name=f"ma{c}")
        nc.sync.dma_start(t[:, :], mat_d[c * 128:(c + 1) * 128, :])
        mat_t.append(t)
        t = const.tile([128, NMEL], F16, tag=f"mb{c}", name=f"mb{c}")
        nc.sync.dma_start(t[:, :], mbt_d[c * 128:(c + 1) * 128, :])
        mbt_t.append(t)
    ident = const.tile([128, 128], F16, tag="id", name="ident")
    nc.sync.dma_start(ident[:, :], idn_d[:, :])
    eps_t = const.tile([128, 1], F32, tag="eps", name="eps")
    nc.vector.memset(eps_t[:, :], EPS)
    hs_t = const.tile([NMEL, 30], F16, tag="hs", name="hs")
    nc.sync.dma_start(hs_t[:, :], hs_d[:, :])
    hd_t = const.tile([NMEL, 30], F16, tag="hd", name="hd")
    nc.sync.dma_start(hd_t[:, :], hd_d[:, :])

    for r in range(ROWS):
        # ---- phase 1: pre-emphasis + on-chip transpose to (441, 1000) fp16
        et = [etp.tile([128, NK], F16, tag=f"et{r}_{c}", name=f"et{r}_{c}") for c in range(4)]
        for (k0, ksz) in ECH:
            e2 = e2p.tile([128, W + 1], F32, tag="e2", name="e2")
            src = bass.AP(xpad, r * (L + 1) + W * k0, [[W, ksz], [1, W + 1]])
            nc.sync.dma_start(e2[0:ksz, :], src)
            tmp = emphp.tile([128, W], F16, tag="tmp", name="tmp")
            nc.scalar.mul(tmp[0:ksz, :], e2[0:ksz, 0:W], 0.97)
            em = emphp.tile([128, W], F16, tag="em", name="em")
            nc.vector.tensor_sub(em[0:ksz, :], e2[0:ksz, 1:W + 1], tmp[0:ksz, :])
            for rb, (rb0, rbsz) in enumerate(RBL):
                ptr = ptrp.tile([128, 128], F16, tag="ptr", name="ptr")
                nc.tensor.transpose(ptr[0:rbsz, 0:ksz], em[0:ksz, rb0:rb0 + rbsz],
                                    ident[0:ksz, 0:ksz])
                nc.scalar.copy(et[rb][0:rbsz, k0:k0 + ksz], ptr[0:rbsz, 0:ksz])

        # ---- phase 2: DFT power -> mel -> log
        lm = lmp.tile([NMEL, T], F16, tag=f"lm{r}", name=f"lm{r}")
        for (f0, fN) in FCH:
            mp = melp.tile([NMEL, 512], F32, tag="mp", name="mp")
            for bc in range(8):
                pre = dftp.tile([128, 512], F32, tag="pre", name="pre")
                pim = dftp.tile([128, 512], F32, tag="pim", name="pim")
                for ki, (r0, sz, a) in enumerate(KCH):
                    rhs = et[r0 // 128][0:sz, f0 + a:f0 + a + fN]
                    nc.tensor.matmul(pre[:, 0:fN], cw_t[ki][0:sz, bc * 128:(bc + 1) * 128],
                                     rhs, start=(ki == 0), stop=(ki == 7))
                for ki, (r0, sz, a) in enumerate(KCH):
                    rhs = et[r0 // 128][0:sz, f0 + a:f0 + a + fN]
                    nc.tensor.matmul(pim[:, 0:fN], sw_t[ki][0:sz, bc * 128:(bc + 1) * 128],
                                     rhs, start=(ki == 0), stop=(ki == 7))
                pa = ppp.tile([128, 512], F16, tag="pa", name="pa")
                nc.scalar.square(pa[:, 0:fN], pre[:, 0:fN])
                pb = ppp.tile([128, 512], F16, tag="pb", name="pb")
                nc.scalar.square(pb[:, 0:fN], pim[:, 0:fN])
                nc.tensor.matmul(mp[0:NMEL, 0:fN], mat_t[bc][:, 0:NMEL], pa[:, 0:fN],
                                 start=(bc == 0), stop=False, skip_group_check=True)
                nc.tensor.matmul(mp[0:NMEL, 0:fN], mbt_t[bc][:, 0:NMEL], pb[:, 0:fN],
                                 start=False, stop=(bc == 7), skip_group_check=True)
            nc.scalar.activation(lm[0:NMEL, f0:f0 + fN], mp[0:NMEL, 0:fN], AF.Ln,
                                 bias=eps_t[0:NMEL, :])

        # ---- phase 3: Haar (as tiny matmuls) / delta / stats
        ca = hop.tile([30, T], F32, tag=f"ca{r}", name=f"ca{r}")
        cd = hop.tile([30, T], F32, tag=f"cd{r}", name=f"cd{r}")
        for (f0, fN) in FCH:
            pca = haarp.tile([30, 512], F32, tag="pca", name="pca")
            nc.tensor.matmul(pca[:, 0:fN], hs_t[:, :], lm[0:NMEL, f0:f0 + fN],
                             start=True, stop=True, skip_group_check=True)
            nc.scalar.copy(ca[:, f0:f0 + fN], pca[:, 0:fN])
            pcd = haarp.tile([30, 512], F32, tag="pcd", name="pcd")
            nc.tensor.matmul(pcd[:, 0:fN], hd_t[:, :], lm[0:NMEL, f0:f0 + fN],
                             start=True, stop=True, skip_group_check=True)
            nc.scalar.copy(cd[:, f0:f0 + fN], pcd[:, 0:fN])
        dl = hop.tile([30, T], F32, tag=f"dl{r}", name=f"dl{r}")
        nc.vector.tensor_sub(dl[:, 1:T - 1], ca[:, 2:T], ca[:, 0:T - 2])
        nc.vector.tensor_sub(dl[:, 0:1], ca[:, 1:2], ca[:, 0:1])
        nc.vector.tensor_sub(dl[:, T - 1:T], ca[:, T - 1:T], ca[:, T - 2:T - 1])

        stats = stp.tile([30, 6], F32, tag=f"stats{r}", name=f"stats{r}")
        for si, feat in enumerate((ca, dl, cd)):
            s1 = stp.tile([30, 1], F32, tag="s1", name="s1")
            nc.vector.tensor_reduce(s1[:, :], feat[:, :], axis=mybir.AxisListType.X,
                                    op=mybir.AluOpType.add)
            nc.vector.tensor_scalar_mul(stats[:, si:si + 1], s1[:, :], 1.0 / (T * SQRT2))
            nm = stp.tile([30, 1], F32, tag="nm", name="nm")
            nc.vector.tensor_scalar_mul(nm[:, :], s1[:, :], -1.0 / T)
            scr = stp.tile([30, T], F32, tag="scr", name="scr")
            sq = stp.tile([30, 1], F32, tag="sq", name="sq")
            nc.scalar.activation(scr[:, :], feat[:, :], AF.Square, bias=nm[:, :],
                                 scale=1.0, accum_out=sq[:, :])
            nc.scalar.activation(stats[:, 3 + si:4 + si], sq[:, :], AF.Sqrt,
                                 scale=1.0 / ((T - 1) * 2.0))
        nc.sync.dma_start(bass.AP(out_d, r * 180, [[1, 180]]), stats[:, :])


_CACHE = {}


def _build():
    if "nc" in _CACHE:
        return _CACHE["nc"]
    nc = bacc.Bacc("TRN2", target_bir_lowering=False, debug=False,
                   enable_asserts=False, num_devices=8)
    xpad = nc.dram_tensor("xpad", [ROWS, L + 1], F32, kind="ExternalInput")
    cw_d = nc.dram_tensor("cw", [FRAME, NB], F16, kind="ExternalInput")
    sw_d = nc.dram_tensor("sw", [FRAME, NB], F16, kind="ExternalInput")
    mat_d = nc.dram_tensor("mat", [NB, NMEL], F16, kind="ExternalInput")
    mbt_d = nc.dram_tensor("mbt", [NB, NMEL], F16, kind="ExternalInput")
    idn_d = nc.dram_tensor("idn", [128, 128], F16, kind="ExternalInput")
    hs_d = nc.dram_tensor("hsum", [NMEL, 30], F16, kind="ExternalInput")
    hd_d = nc.dram_tensor("hdif", [NMEL, 30], F16, kind="ExternalInput")
    out_d = nc.dram_tensor("out", [ROWS, 180], F32, kind="ExternalOutput")
    with tile.TileContext(nc) as tc, ExitStack() as ctx:
        _body(ctx, tc, xpad, cw_d, sw_d, mat_d, mbt_d, idn_d, hs_d, hd_d, out_d)
    nc.compile()
    _CACHE["nc"] = nc
    return nc


def make_in_maps(waveform: np.ndarray, mel_filters: np.ndarray):
    cw, sw, mat, mbt, idn, hsum, hdif = _host_constants(mel_filters)
    in_maps = []
    for core in range(8):
        rows = waveform[ROWS * core:ROWS * (core + 1)]
        xpad = np.zeros((ROWS, L + 1), np.float32)
        xpad[:, 1:] = rows
        in_maps.append({"xpad": xpad, "cw": cw, "sw": sw, "mat": mat,
                        "mbt": mbt, "idn": idn, "hsum": hsum, "hdif": hdif})
    return in_maps


def gather_out(results):
    # device rows are packed [mel_idx, stat]; reorder to [stat, mel_idx]
    full = np.concatenate([results[c]["out"] for c in range(8)], axis=0)
    return np.ascontiguousarray(
        full.reshape(B, 30, 6).transpose(0, 2, 1).reshape(B, 180)).astype(np.float32)


def run(waveform, mel_filters, trace=False):
    nc = _build()
    in_maps = make_in_maps(np.asarray(waveform, np.float32),
                           np.asarray(mel_filters, np.float32))
    res = run_bass_kernel_spmd(nc, in_maps, core_ids=list(range(8)), trace=trace)
    return gather_out(res.results), res


def kernel(waveform: np.ndarray, mel_filters: np.ndarray) -> np.ndarray:
    out, _ = run(waveform, mel_filters, trace=False)
    return out



# revision 11
# speedup vs baseline: 1.3668x; 1.3668x over previous
"""MFDWC feature extractor as a Bass/Tile kernel for TRN2 (8 NeuronCores).

Pipeline (per batch row): pre-emphasis -> framing (999 frames x 882 samples,
hop 441) -> Hamming window -> rFFT(2048) power spectrum -> mel (60) -> log ->
Haar DWT -> delta -> mean/std over time -> 180 features.

Device mapping:
  - Data parallel: 16 batch rows -> 2 rows per core on 8 cores.
  - rFFT is computed as DFT matmuls (cos / sin matrices, window folded in)
    in fp8e4 with DoubleRow perf mode: each matmul contracts 2x128 frame
    samples per pass at double pump rate (157 TF/s), fp32 PSUM accumulation.
    The whole 882-sample contraction takes 4 DoubleRow matmuls per 128-bin
    block (3 full pairs + one 57-row tail pair).
  - The waveform (host-prescaled by 1/sqrt(128) so power spectra stay in
    fp8 range) is reshaped on-chip to (441, 1000) fp8: pre-emphasis is a
    single fused DVE op, then PE transposes of 441-wide chunks. Frames
    appear as overlapping column views; the DoubleRow k-subtile pair
    (frame first/second half) is an overlapping stride-1 column view too.
  - Bins packing: cos matmul covers bins 0..1023; the sin matrix's bin-0
    column (all zeros) instead carries the Nyquist cos column, and the mel
    matrix is adjusted to match, so all 1025 power bins are covered.
  - power -> mel is one DoubleRow fp8 matmul per bin block (cos^2 and sin^2
    power halves are the two k-subtiles); log/Haar/delta/stats run on
    ACT/DVE engines. log folds the 128x power rescale via Ln's input scale.
"""

import math
from contextlib import ExitStack

import ml_dtypes
import numpy as np

import concourse.bass as bass
import concourse.bacc as bacc
import concourse.mybir as mybir
import concourse.tile as tile
from concourse.bass_utils import run_bass_kernel_spmd

F32 = mybir.dt.float32
F16 = mybir.dt.float16
F8 = mybir.dt.float8e4
AF = mybir.ActivationFunctionType
ALU = mybir.AluOpType
DR = mybir.MatmulPerfMode.DoubleRow
E4NP = ml_dtypes.float8_e4m3  # TRN-compatible e4m3 (max +-240)

B = 16               # batch
L = 441000           # samples per row
W = 441              # hop; also chunk width
NK = 1000            # number of 441-sample chunks per row
FRAME = 882          # frame length
T = 999              # frames per row
NB = 1024            # matmul bins (bins 0..1023; Nyquist packed into sin col 0)
NMEL = 60
ROWS = 2             # batch rows per core
EPS = 1e-10
SQRT2 = math.sqrt(2.0)
ACT_SCALE = 1.0 / math.sqrt(128.0)   # waveform prescale; power ends up /128
PSCALE = 128.0                       # undone inside Ln via input scale

# contraction row-blocks over the 441 samples per chunk; the DoubleRow
# k-subtile dim covers the two frame halves (sample j = 441*k + r0 + i)
KCH = [(0, 128), (128, 128), (256, 128), (384, 57)]
# chunks over the NK=1000 waveform columns
ECH = [(k * 128, min(128, NK - k * 128)) for k in range(8)]
# frame chunks (PSUM free-dim <= 512 fp32)
FCH = [(0, 512), (512, 487)]


def _q8(x):
    return np.clip(np.asarray(x, np.float64), -240, 240).astype(E4NP)


def _host_constants(mel_filters: np.ndarray):
    """DFT / mel matrices with window folded in, packed for DoubleRow fp8."""
    j = np.arange(FRAME, dtype=np.float64)
    ham = np.hamming(FRAME).astype(np.float64)
    ang = 2.0 * np.pi * np.outer(j, np.arange(NB, dtype=np.float64)) / 2048.0
    cwf = ham[:, None] * np.cos(ang)                    # (882, 1024)
    swf = ham[:, None] * np.sin(ang)
    swf[:, 0] = ham * np.cos(np.pi * j)                 # Nyquist cos col
    # per row-block rb: [128, 2, 1024] -> [128, 2048]; row p col a*1024+m
    # holds weight for sample j = 441*a + r0 + p, bin m
    cw8 = np.zeros((512, 2048), E4NP)
    sw8 = np.zeros((512, 2048), E4NP)
    for rb, (r0, sz) in enumerate(KCH):
        for a in (0, 1):
            cw8[rb * 128:rb * 128 + sz, a * NB:(a + 1) * NB] = \
                _q8(cwf[441 * a + r0:441 * a + r0 + sz])
            sw8[rb * 128:rb * 128 + sz, a * NB:(a + 1) * NB] = \
                _q8(swf[441 * a + r0:441 * a + r0 + sz])
    m = np.asarray(mel_filters, np.float64)             # (60, 1025)
    melw8 = np.zeros((NB, 256), E4NP)                   # [bin, k*128+mel] (padded)
    melw8[:, 0:NMEL] = _q8(m[:, 0:NB].T)                # k=0: cos^2 powers
    melw8[:, 128:128 + NMEL] = _q8(
        np.concatenate([m[:, NB:NB + 1], m[:, 1:NB]], axis=1).T)  # k=1: sin^2
    idn8 = np.eye(128, dtype=np.float16)
    hstack = np.zeros((NMEL, 60), np.float16)           # [haar-sum | haar-diff]
    for i in range(30):
        hstack[2 * i, i] = 1.0
        hstack[2 * i + 1, i] = 1.0
        hstack[2 * i, 30 + i] = 1.0
        hstack[2 * i + 1, 30 + i] = -1.0
    return cw8, sw8, melw8, idn8, hstack


def _view(t, offset, dims):
    """Overlapping/strided AP view on a tile (dims = [[stride, size], ...])."""
    return bass.AP(t[0:1, 0:1].tensor, offset, dims)


def _body(ctx: ExitStack, tc, xpad, cw_d, sw_d, melw_d, idn_d, hst_d, out_d):
    nc = tc.nc

    const = ctx.enter_context(tc.tile_pool(name="const", bufs=1))
    e2p = ctx.enter_context(tc.tile_pool(name="e2", bufs=3))
    emp = ctx.enter_context(tc.tile_pool(name="emph", bufs=3))
    etp = ctx.enter_context(tc.tile_pool(name="et", bufs=1))
    ptrp = ctx.enter_context(tc.tile_pool(name="ptr", bufs=1, space="PSUM"))
    dftp = ctx.enter_context(tc.tile_pool(name="dft", bufs=1, space="PSUM"))
    melp = ctx.enter_context(tc.tile_pool(name="mel", bufs=1, space="PSUM"))
    haarp = ctx.enter_context(tc.tile_pool(name="haar", bufs=1, space="PSUM"))
    ppp = ctx.enter_context(tc.tile_pool(name="pp", bufs=2))
    lmp = ctx.enter_context(tc.tile_pool(name="lm", bufs=1))
    hop = ctx.enter_context(tc.tile_pool(name="ho", bufs=1))
    stp = ctx.enter_context(tc.tile_pool(name="st", bufs=2))

    # constants
    cw_t, sw_t = [], []
    for rb, (r0, sz) in enumerate(KCH):
        t = const.tile([128, 2048], F8, tag=f"cw{rb}", name=f"cw{rb}")
        nc.sync.dma_start(t[0:sz, :], cw_d[rb * 128:rb * 128 + sz, :])
        cw_t.append(t)
        t = const.tile([128, 2048], F8, tag=f"sw{rb}", name=f"sw{rb}")
        nc.sync.dma_start(t[0:sz, :], sw_d[rb * 128:rb * 128 + sz, :])
        sw_t.append(t)
    melw_t = []
    for c in range(8):
        t = const.tile([128, 256], F8, tag=f"mw{c}", name=f"mw{c}")
        nc.sync.dma_start(t[:, :], melw_d[c * 128:(c + 1) * 128, :])
        melw_t.append(t)
    ident16 = const.tile([128, 128], F16, tag="id", name="ident16")
    nc.sync.dma_start(ident16[:, :], idn_d[:, :])
    hst = const.tile([NMEL, 60], F16, tag="hs", name="hst")
    nc.sync.dma_start(hst[:, :], hst_d[:, :])
    eps_t = const.tile([128, 1], F32, tag="eps", name="eps")
    nc.vector.memset(eps_t[:, :], EPS)

    for r in range(ROWS):
        # ---- phase 1: fused pre-emphasis+quantize, transpose to (441,1000) fp8
        et = [etp.tile([128, 2 * NK], F8, tag=f"et{r}_{c}", name=f"et{r}_{c}")
              for c in range(4)]
        for (k0, ksz) in ECH:
            e2 = e2p.tile([128, W + 1], F16, tag="e2", name="e2")
            src = bass.AP(xpad, r * (L + 1) + W * k0, [[W, ksz], [1, W + 1]])
            nc.sync.dma_start(e2[0:ksz, :], src)
            em = emp.tile([128, W], F16, tag="em", name="em")
            nc.vector.scalar_tensor_tensor(
                em[0:ksz, :], e2[0:ksz, 0:W], -0.97, e2[0:ksz, 1:W + 1],
                op0=ALU.mult, op1=ALU.add)
            for rb, (r0, rbsz) in enumerate(KCH):
                ptr = ptrp.tile([128, 128], F16, tag="ptr", name="ptr")
                nc.tensor.transpose(ptr[0:rbsz, 0:ksz], em[0:ksz, r0:r0 + rbsz],
                                    ident16[0:ksz, 0:ksz])
                # slot 0: chunk c at col c; slot 1: chunk c at col c-1
                va = et[rb][0:rbsz, k0:k0 + ksz]
                if k0 == 0:
                    vb = et[rb][0:rbsz, NK:NK + ksz - 1]
                    pb = ptr[0:rbsz, 1:ksz]
                else:
                    vb = et[rb][0:rbsz, NK + k0 - 1:NK + k0 + ksz - 1]
                    pb = ptr[0:rbsz, 0:ksz]
                if rb % 2 == 0:
                    nc.vector.tensor_copy(va, ptr[0:rbsz, 0:ksz])
                    nc.scalar.copy(vb, pb)
                else:
                    nc.scalar.copy(va, ptr[0:rbsz, 0:ksz])
                    nc.vector.tensor_copy(vb, pb)

        # ---- phase 2: DoubleRow DFT power -> mel -> log
        lm = lmp.tile([NMEL, T], F16, tag=f"lm{r}", name=f"lm{r}")
        mp = [melp.tile([128, 512], F32, tag=f"mp{fi}", name=f"mp{fi}_{r}")
              for fi in range(2)]
        for bc in range(8):
            pre = [dftp.tile([128, 512], F32, tag=f"pre{fi}", name=f"pre{fi}")
                   for fi in range(2)]
            pim = [dftp.tile([128, 512], F32, tag=f"pim{fi}", name=f"pim{fi}")
                   for fi in range(2)]
            for dst, w_t in ((pre, cw_t), (pim, sw_t)):
                for rb, (r0, sz) in enumerate(KCH):
                    w = _view(w_t[rb], bc * 128, [[2048, sz], [1024, 2], [1, 128]])
                    for fi, (f0, fN) in enumerate(FCH):
                        rhs = _view(et[rb], f0,
                                    [[2 * NK, sz], [NK, 2], [1, fN]])
                        nc.tensor.matmul(dst[fi][:, 0:fN], w, rhs,
                                         start=(rb == 0), stop=(rb == 3),
                                         perf_mode=DR, skip_group_check=True)
            for fi, (f0, fN) in enumerate(FCH):
                pp = ppp.tile([128, 1024], F8, tag=f"pp{fi}", name=f"pp{fi}")
                nc.scalar.square(pp[0:128, 0:fN], pre[fi][:, 0:fN])
                pb16 = ppp.tile([128, 512], F16, tag=f"pb{fi}", name=f"pb{fi}")
                nc.vector.tensor_copy(pb16[:, 0:fN], pim[fi][:, 0:fN])
                nc.vector.tensor_mul(pp[0:128, 512:512 + fN],
                                     pb16[:, 0:fN], pb16[:, 0:fN])
                wm = _view(melw_t[bc], 0, [[256, 128], [128, 2], [1, 128]])
                rhsm = _view(pp, 0, [[1024, 128], [512, 2], [1, fN]])
                nc.tensor.matmul(mp[fi][0:128, 0:fN], wm, rhsm,
                                 start=(bc == 0), stop=(bc == 7),
                                 perf_mode=DR, skip_group_check=True)
        for fi, (f0, fN) in enumerate(FCH):
            nc.scalar.activation(lm[0:NMEL, f0:f0 + fN], mp[fi][0:NMEL, 0:fN],
                                 AF.Ln, bias=eps_t[0:NMEL, :], scale=PSCALE)

        # ---- phase 3: Haar (tiny matmuls) / delta / stats
        ca = hop.tile([30, T], F32, tag=f"ca{r}", name=f"ca{r}")
        cd = hop.tile([30, T], F32, tag=f"cd{r}", name=f"cd{r}")
        for (f0, fN) in FCH:
            pca = haarp.tile([30, 512], F32, tag="hp", name="pca")
            nc.tensor.matmul(pca[:, 0:fN], hst[:, 0:30], lm[0:NMEL, f0:f0 + fN],
                             start=True, stop=True, skip_group_check=True)
            nc.scalar.copy(ca[:, f0:f0 + fN], pca[:, 0:fN])
            pcd = haarp.tile([30, 512], F32, tag="hp", name="pcd")
            nc.tensor.matmul(pcd[:, 0:fN], hst[:, 30:60], lm[0:NMEL, f0:f0 + fN],
                             start=True, stop=True, skip_group_check=True)
            nc.scalar.copy(cd[:, f0:f0 + fN], pcd[:, 0:fN])
        dl = hop.tile([30, T], F32, tag=f"dl{r}", name=f"dl{r}")
        nc.vector.tensor_sub(dl[:, 1:T - 1], ca[:, 2:T], ca[:, 0:T - 2])
        nc.vector.tensor_sub(dl[:, 0:1], ca[:, 1:2], ca[:, 0:1])
        nc.vector.tensor_sub(dl[:, T - 1:T], ca[:, T - 1:T], ca[:, T - 2:T - 1])

        stats = stp.tile([30, 6], F32, tag=f"stats{r}", name=f"stats{r}")
        for si, feat in enumerate((ca, dl, cd)):
            s1 = stp.tile([30, 1], F32, tag="s1", name="s1")
            nc.vector.tensor_reduce(s1[:, :], feat[:, :], axis=mybir.AxisListType.X,
                                    op=mybir.AluOpType.add)
            nc.vector.tensor_scalar_mul(stats[:, si:si + 1], s1[:, :], 1.0 / (T * SQRT2))
            nm = stp.tile([30, 1], F32, tag="nm", name="nm")
            nc.vector.tensor_scalar_mul(nm[:, :], s1[:, :], -1.0 / T)
            scr = stp.tile([30, T], F32, tag="scr", name="scr")
            sq = stp.tile([30, 1], F32, tag="sq", name="sq")
            nc.scalar.activation(scr[:, :], feat[:, :], AF.Square, bias=nm[:, :],
                                 scale=1.0, accum_out=sq[:, :])
            nc.scalar.activation(stats[:, 3 + si:4 + si], sq[:, :], AF.Sqrt,
                                 scale=1.0 / ((T - 1) * 2.0))
        nc.sync.dma_start(bass.AP(out_d, r * 180, [[1, 180]]), stats[:, :])


_CACHE = {}


def _build():
    if "nc" in _CACHE:
        return _CACHE["nc"]
    nc = bacc.Bacc("TRN2", target_bir_lowering=False, debug=False,
                   enable_asserts=False, num_devices=8)
    xpad = nc.dram_tensor("xpad", [ROWS, L + 1], F16, kind="ExternalInput")
    cw_d = nc.dram_tensor("cw", [512, 2048], F8, kind="ExternalInput")
    sw_d = nc.dram_tensor("sw", [512, 2048], F8, kind="ExternalInput")
    melw_d = nc.dram_tensor("melw", [NB, 256], F8, kind="ExternalInput")
    idn_d = nc.dram_tensor("idn", [128, 128], F16, kind="ExternalInput")
    hst_d = nc.dram_tensor("hst", [NMEL, 60], F16, kind="ExternalInput")
    out_d = nc.dram_tensor("out", [ROWS, 180], F32, kind="ExternalOutput")
    with tile.TileContext(nc) as tc, ExitStack() as ctx:
        _body(ctx, tc, xpad, cw_d, sw_d, melw_d, idn_d, hst_d, out_d)
    nc.compile()
    _CACHE["nc"] = nc
    return nc


def make_in_maps(waveform: np.ndarray, mel_filters: np.ndarray):
    cw8, sw8, melw8, idn8, hstack = _host_constants(mel_filters)
    scaled = (waveform.astype(np.float32) * ACT_SCALE).astype(np.float16)
    in_maps = []
    for core in range(8):
        rows = scaled[ROWS * core:ROWS * (core + 1)]
        xpad = np.zeros((ROWS, L + 1), np.float16)
        xpad[:, 1:] = rows
        in_maps.append({"xpad": xpad, "cw": cw8, "sw": sw8, "melw": melw8,
                        "idn": idn8, "hst": hstack})
    return in_maps


def gather_out(results):
    # device rows are packed [mel_idx, stat]; reorder to [stat, mel_idx]
    full = np.concatenate([results[c]["out"] for c in range(8)], axis=0)
    return np.ascontiguousarray(
        full.reshape(B, 30, 6).transpose(0, 2, 1).reshape(B, 180)).astype(np.float32)


def run(waveform, mel_filters, trace=False):
    nc = _build()
    in_maps = make_in_maps(np.asarray(waveform, np.float32),
                           np.asarray(mel_filters, np.float32))
    res = run_bass_kernel_spmd(nc, in_maps, core_ids=list(range(8)), trace=trace)
    return gather_out(res.results), res


def kernel(waveform: np.ndarray, mel_filters: np.ndarray) -> np.ndarray:
    out, _ = run(waveform, mel_filters, trace=False)
    return out


# revision 14
# speedup vs baseline: 2.0774x; 1.5199x over previous
"""MFDWC feature extractor as a Bass/Tile kernel for TRN2 (8 NeuronCores).

Pipeline (per batch row): pre-emphasis -> framing (999 frames x 882 samples,
hop 441) -> Hamming window -> rFFT(2048) power spectrum -> mel (60) -> log ->
Haar DWT -> delta -> mean/std over time -> 180 features.

Device mapping:
  - Data parallel: 16 batch rows -> 2 rows per core on 8 cores.
  - Pre-emphasis, 1/sqrt(128) scaling (keeps power spectra in fp8 range) and
    the fp8e4 cast run on the host; the device receives the emphasized signal
    packed as (1000 chunks x 442 bytes) = overlapping 441-sample hops.
  - Phase 1 is pure DMA: two XBAR DMA-transposes per row move the signal
    into SBUF as (sample-pair word, chunk) fp16 words; each SBUF partition
    holds an (even, odd) fp8 sample pair of every chunk.
  - rFFT is DFT matmuls (cos/sin matrices, Hamming window folded in) in fp8
    DoubleRow mode: the k-subtile pair = (even, odd) samples, so each matmul
    contracts 2x(up to 128) samples per pass at double-pump rate. Frame halves
    (hop offset) are plain +2-byte column offsets - no overlapping APs.
  - Bins packing: cos matmul covers bins 0..1023; the sin matrix's bin-0
    column (all zeros) instead carries the Nyquist cos column, and the mel
    matrix is adjusted to match, so all 1025 power bins are covered.
  - power -> mel is one DoubleRow fp8 matmul per bin block (cos^2 / sin^2
    halves are the two k-subtiles). Squares are split across Scalar (cos),
    GpSimd (PSUM evacuate) and Vector (square) engines. log folds the 128x
    power rescale via Ln's input scale; Haar/delta/stats close out each row.
"""

import math
from contextlib import ExitStack

import ml_dtypes
import numpy as np

import concourse.bass as bass
import concourse.bacc as bacc
import concourse.mybir as mybir
import concourse.tile as tile
from concourse.bass_utils import run_bass_kernel_spmd

F32 = mybir.dt.float32
F16 = mybir.dt.float16
F8 = mybir.dt.float8e4
AF = mybir.ActivationFunctionType
DR = mybir.MatmulPerfMode.DoubleRow
E4NP = ml_dtypes.float8_e4m3  # TRN-compatible e4m3 (max +-240)

B = 16               # batch
L = 441000           # samples per row
W = 441              # hop; chunk stride (chunks padded to 442 bytes)
NK = 1000            # chunks per row
NKP = 1008           # chunk rows padded for XBAR transpose (16 | NKP)
WP = 256             # fp16 words per chunk row, padded for XBAR (128 | WP)
FRAME = 882          # frame length
T = 999              # frames per row
NB = 1024            # matmul bins (bins 0..1023; Nyquist packed into sin col 0)
NMEL = 60
ROWS = 2             # batch rows per core
EPS = 1e-10
SQRT2 = math.sqrt(2.0)
ACT_SCALE = 1.0 / math.sqrt(128.0)   # waveform prescale; power ends up /128
PSCALE = 128.0                       # undone inside Ln via input scale

# contraction groups: (frame half a, first sample-pair word, partitions)
KCH = [(0, 0, 128), (0, 128, 93), (1, 0, 128), (1, 128, 93)]
# frame chunks (PSUM free-dim <= 512 fp32)
FCH = [(0, 512), (512, 487)]


def _q8(x):
    return np.clip(np.asarray(x, np.float64), -240, 240).astype(E4NP)


def _host_constants(mel_filters: np.ndarray):
    """DFT / mel matrices with window folded in, packed for DoubleRow fp8."""
    j = np.arange(FRAME, dtype=np.float64)
    ham = np.hamming(FRAME).astype(np.float64)
    ang = 2.0 * np.pi * np.outer(j, np.arange(NB, dtype=np.float64)) / 2048.0
    cwf = ham[:, None] * np.cos(ang)                    # (882, 1024)
    swf = ham[:, None] * np.sin(ang)
    swf[:, 0] = ham * np.cos(np.pi * j)                 # Nyquist cos col
    # group g row p col k*1024+m holds the weight for within-chunk sample
    # i = 2*(wbase+p)+k of frame half a (frame sample j = 441*a + i);
    # i == 441 addresses the pad byte -> weight 0
    cw8 = np.zeros((512, 2048), E4NP)
    sw8 = np.zeros((512, 2048), E4NP)
    for g, (a, wbase, psz) in enumerate(KCH):
        for k in (0, 1):
            i = 2 * (wbase + np.arange(psz)) + k
            valid = i <= 440
            rows_c = np.zeros((psz, NB))
            rows_s = np.zeros((psz, NB))
            rows_c[valid] = cwf[441 * a + i[valid], :]
            rows_s[valid] = swf[441 * a + i[valid], :]
            cw8[g * 128:g * 128 + psz, k * NB:(k + 1) * NB] = _q8(rows_c)
            sw8[g * 128:g * 128 + psz, k * NB:(k + 1) * NB] = _q8(rows_s)
    m = np.asarray(mel_filters, np.float64)             # (60, 1025)
    melw8 = np.zeros((NB, 256), E4NP)                   # [bin, k*128+mel] (padded)
    melw8[:, 0:NMEL] = _q8(m[:, 0:NB].T)                # k=0: cos^2 powers
    melw8[:, 128:128 + NMEL] = _q8(
        np.concatenate([m[:, NB:NB + 1], m[:, 1:NB]], axis=1).T)  # k=1: sin^2
    hstack = np.zeros((NMEL, 60), np.float16)           # [haar-sum | haar-diff]
    for i2 in range(30):
        hstack[2 * i2, i2] = 1.0
        hstack[2 * i2 + 1, i2] = 1.0
        hstack[2 * i2, 30 + i2] = 1.0
        hstack[2 * i2 + 1, 30 + i2] = -1.0
    return cw8, sw8, melw8, hstack


def _view(t, offset, dims):
    """Strided AP view on a tile (dims = [[stride, size], ...])."""
    return bass.AP(t[0:1, 0:1].tensor, offset, dims)


def _body(ctx: ExitStack, tc, xp_d, cw_d, sw_d, melw_d, hst_d, out_d):
    nc = tc.nc

    const = ctx.enter_context(tc.tile_pool(name="const", bufs=1))
    etp = ctx.enter_context(tc.tile_pool(name="et", bufs=1))
    dftp = ctx.enter_context(tc.tile_pool(name="dft", bufs=1, space="PSUM"))
    melp = ctx.enter_context(tc.tile_pool(name="mel", bufs=1, space="PSUM"))
    haarp = ctx.enter_context(tc.tile_pool(name="haar", bufs=1, space="PSUM"))
    ppp = ctx.enter_context(tc.tile_pool(name="pp", bufs=2))
    lmp = ctx.enter_context(tc.tile_pool(name="lm", bufs=1))
    hop = ctx.enter_context(tc.tile_pool(name="ho", bufs=1))
    stp = ctx.enter_context(tc.tile_pool(name="st", bufs=2))

    # constants
    cw_t, sw_t = [], []
    for g, (a, wbase, psz) in enumerate(KCH):
        t = const.tile([128, 2048], F8, tag=f"cw{g}", name=f"cw{g}")
        nc.sync.dma_start(t[0:psz, :], cw_d[g * 128:g * 128 + psz, :])
        cw_t.append(t)
        t = const.tile([128, 2048], F8, tag=f"sw{g}", name=f"sw{g}")
        nc.sync.dma_start(t[0:psz, :], sw_d[g * 128:g * 128 + psz, :])
        sw_t.append(t)
    melw_t = []
    for c in range(8):
        t = const.tile([128, 256], F8, tag=f"mw{c}", name=f"mw{c}")
        nc.sync.dma_start(t[:, :], melw_d[c * 128:(c + 1) * 128, :])
        melw_t.append(t)
    hst = const.tile([NMEL, 60], F16, tag="hs", name="hst")
    nc.sync.dma_start(hst[:, :], hst_d[:, :])
    eps_t = const.tile([128, 1], F32, tag="eps", name="eps")
    nc.vector.memset(eps_t[:, :], EPS)

    for r in range(ROWS):
        # ---- phase 1: two XBAR DMA-transposes bring the emphasized signal
        # into SBUF as (sample-pair word, chunk) fp16 words
        et = [etp.tile([128, NKP], F16, tag=f"et{r}_{blk}", name=f"et{r}_{blk}")
              for blk in range(2)]
        nc.sync.dma_start_transpose(
            et[0][0:128, :], xp_d[r * NKP:(r + 1) * NKP, 0:128])
        nc.sync.dma_start_transpose(
            et[1][0:128, :], xp_d[r * NKP:(r + 1) * NKP, 128:256])

        # ---- phase 2: DoubleRow DFT power -> mel -> log
        lm = lmp.tile([NMEL, T], F16, tag=f"lm{r}", name=f"lm{r}")
        mp = [melp.tile([128, 512], F32, tag=f"mp{fi}", name=f"mp{fi}_{r}")
              for fi in range(2)]
        for bc in range(8):
            pre = [dftp.tile([128, 512], F32, tag=f"pre{fi}", name=f"pre{fi}")
                   for fi in range(2)]
            pim = [dftp.tile([128, 512], F32, tag=f"pim{fi}", name=f"pim{fi}")
                   for fi in range(2)]
            for dst, w_t in ((pre, cw_t), (pim, sw_t)):
                for g, (a, wbase, psz) in enumerate(KCH):
                    blk = 0 if wbase == 0 else 1
                    w = _view(w_t[g], bc * 128, [[2048, psz], [1024, 2], [1, 128]])
                    x8 = et[blk][0:psz, 0:NKP].bitcast(F8)
                    for fi, (f0, fN) in enumerate(FCH):
                        rhs = bass.AP(x8.tensor, 2 * (f0 + a),
                                      [[2 * NKP, psz], [1, 2], [2, fN]])
                        nc.tensor.matmul(dst[fi][:, 0:fN], w, rhs,
                                         start=(g == 0), stop=(g == 3),
                                         perf_mode=DR, skip_group_check=True)
            for fi, (f0, fN) in enumerate(FCH):
                pp = ppp.tile([128, 1024], F8, tag=f"pp{fi}", name=f"pp{fi}")
                nc.scalar.square(pp[0:128, 0:fN], pre[fi][:, 0:fN])
                pb16 = ppp.tile([128, 512], F16, tag=f"pb{fi}", name=f"pb{fi}")
                nc.vector.tensor_copy(pb16[:, 0:fN], pim[fi][:, 0:fN])
                nc.gpsimd.tensor_mul(pp[0:128, 512:512 + fN],
                                     pb16[:, 0:fN], pb16[:, 0:fN])
                wm = _view(melw_t[bc], 0, [[256, 128], [128, 2], [1, 128]])
                rhsm = _view(pp, 0, [[1024, 128], [512, 2], [1, fN]])
                nc.tensor.matmul(mp[fi][0:128, 0:fN], wm, rhsm,
                                 start=(bc == 0), stop=(bc == 7),
                                 perf_mode=DR, skip_group_check=True)
        for fi, (f0, fN) in enumerate(FCH):
            nc.scalar.activation(lm[0:NMEL, f0:f0 + fN], mp[fi][0:NMEL, 0:fN],
                                 AF.Ln, bias=eps_t[0:NMEL, :], scale=PSCALE)

        # ---- phase 3: Haar (tiny matmuls) / delta / stats
        ca = hop.tile([30, T], F32, tag=f"ca{r}", name=f"ca{r}")
        cd = hop.tile([30, T], F32, tag=f"cd{r}", name=f"cd{r}")
        for (f0, fN) in FCH:
            pca = haarp.tile([30, 512], F32, tag="hp", name="pca")
            nc.tensor.matmul(pca[:, 0:fN], hst[:, 0:30], lm[0:NMEL, f0:f0 + fN],
                             start=True, stop=True, skip_group_check=True)
            nc.scalar.copy(ca[:, f0:f0 + fN], pca[:, 0:fN])
            pcd = haarp.tile([30, 512], F32, tag="hp", name="pcd")
            nc.tensor.matmul(pcd[:, 0:fN], hst[:, 30:60], lm[0:NMEL, f0:f0 + fN],
                             start=True, stop=True, skip_group_check=True)
            nc.scalar.copy(cd[:, f0:f0 + fN], pcd[:, 0:fN])
        dl = hop.tile([30, T], F32, tag=f"dl{r}", name=f"dl{r}")
        nc.vector.tensor_sub(dl[:, 1:T - 1], ca[:, 2:T], ca[:, 0:T - 2])
        nc.vector.tensor_sub(dl[:, 0:1], ca[:, 1:2], ca[:, 0:1])
        nc.vector.tensor_sub(dl[:, T - 1:T], ca[:, T - 1:T], ca[:, T - 2:T - 1])

        stats = stp.tile([30, 6], F32, tag=f"stats{r}", name=f"stats{r}")
        for si, feat in enumerate((ca, dl, cd)):
            s1 = stp.tile([30, 1], F32, tag="s1", name="s1")
            nc.vector.tensor_reduce(s1[:, :], feat[:, :], axis=mybir.AxisListType.X,
                                    op=mybir.AluOpType.add)
            nc.vector.tensor_scalar_mul(stats[:, si:si + 1], s1[:, :], 1.0 / (T * SQRT2))
            nm = stp.tile([30, 1], F32, tag="nm", name="nm")
            nc.vector.tensor_scalar_mul(nm[:, :], s1[:, :], -1.0 / T)
            scr = stp.tile([30, T], F32, tag="scr", name="scr")
            sq = stp.tile([30, 1], F32, tag="sq", name="sq")
            nc.scalar.activation(scr[:, :], feat[:, :], AF.Square, bias=nm[:, :],
                                 scale=1.0, accum_out=sq[:, :])
            nc.scalar.activation(stats[:, 3 + si:4 + si], sq[:, :], AF.Sqrt,
                                 scale=1.0 / ((T - 1) * 2.0))
        nc.sync.dma_start(bass.AP(out_d, r * 180, [[1, 180]]), stats[:, :])


_CACHE = {}


def _build():
    if "nc" in _CACHE:
        return _CACHE["nc"]
    nc = bacc.Bacc("TRN2", target_bir_lowering=False, debug=False,
                   enable_asserts=False, num_devices=8)
    xp_d = nc.dram_tensor("xp", [ROWS * NKP, WP], F16, kind="ExternalInput")
    cw_d = nc.dram_tensor("cw", [512, 2048], F8, kind="ExternalInput")
    sw_d = nc.dram_tensor("sw", [512, 2048], F8, kind="ExternalInput")
    melw_d = nc.dram_tensor("melw", [NB, 256], F8, kind="ExternalInput")
    hst_d = nc.dram_tensor("hst", [NMEL, 60], F16, kind="ExternalInput")
    out_d = nc.dram_tensor("out", [ROWS, 180], F32, kind="ExternalOutput")
    with tile.TileContext(nc) as tc, ExitStack() as ctx:
        _body(ctx, tc, xp_d, cw_d, sw_d, melw_d, hst_d, out_d)
    nc.compile()
    _CACHE["nc"] = nc
    return nc


def make_in_maps(waveform: np.ndarray, mel_filters: np.ndarray):
    cw8, sw8, melw8, hstack = _host_constants(mel_filters)
    wav = waveform.astype(np.float32)
    emph = np.concatenate([wav[:, :1], wav[:, 1:] - 0.97 * wav[:, :-1]],
                          axis=1) * ACT_SCALE
    e8 = np.clip(emph, -240, 240).astype(E4NP)          # (B, 441000)
    epad = np.concatenate([e8, np.zeros((B, 1), E4NP)], axis=1)
    # chunk-major pack: (B, NKP chunk rows, 512 bytes), chunks overlap by 1;
    # rows/cols padded for the XBAR transpose tiling
    strided = np.lib.stride_tricks.as_strided(
        epad, (B, NK, W + 1),
        (epad.strides[0], W * epad.strides[1], epad.strides[1]))
    xp = np.zeros((B, NKP, 2 * WP), E4NP)
    xp[:, 0:NK, 0:W + 1] = strided
    in_maps = []
    for core in range(8):
        rows = np.ascontiguousarray(xp[ROWS * core:ROWS * (core + 1)])
        xp16 = rows.reshape(ROWS * NKP, 2 * WP).view(np.float16)  # [2016, 256]
        in_maps.append({"xp": xp16, "cw": cw8, "sw": sw8, "melw": melw8,
                        "hst": hstack})
    return in_maps


def gather_out(results):
    # device rows are packed [mel_idx, stat]; reorder to [stat, mel_idx]
    full = np.concatenate([results[c]["out"] for c in range(8)], axis=0)
    return np.ascontiguousarray(
        full.reshape(B, 30, 6).transpose(0, 2, 1).reshape(B, 180)).astype(np.float32)


def run(waveform, mel_filters, trace=False):
    nc = _build()
    in_maps = make_in_maps(np.asarray(waveform, np.float32),
                           np.asarray(mel_filters, np.float32))
    res = run_bass_kernel_spmd(nc, in_maps, core_ids=list(range(8)), trace=trace)
    return gather_out(res.results), res


def kernel(waveform: np.ndarray, mel_filters: np.ndarray) -> np.ndarray:
    out, _ = run(waveform, mel_filters, trace=False)
    return out
